# revision 40
# baseline (speedup 1.0000x reference)
"""AdaptConv2d Trainium2 kernel: host-routed, balanced 8-core sparse conv.

The gates (layer LSTM gate + channel gate) are tiny compared to the main
conv, but they are data-dependent and the active samples cluster badly
under a contiguous batch split (SPMD time = slowest core).  So:

  Host: computes both gates exactly in fp64-tailed numpy (margins on the
        binary decisions are ~1e-3; fp32/fp64 host math is ~1e-6 off the
        fp32 jax reference, so decisions match).  Pass-through channels
        (out = x) are assembled on host.  Only the ~17 active samples'
        ~116 selected channels need conv on device.

  Device: a fully static SPMD program - no If/For_i/values_load/indirect
        DMA.  Work is chunked at (sample, 8-output-row) granularity and
        packed into an identical per-core slot template (e.g. [7,7,1] =
        15 chunks/core for 119 total chunks), so all 8 cores finish
        together.  The conv runs as 1-pass fp8e4m3 DoubleRow matmuls
        (Wh.Xh with WSCALE*w and x round-to-nearest fp8; end-to-end rel
        err 1.817e-2 vs the 2e-2 gate, deterministic): DR contracts both
        128-cin blocks per instruction, halving the matmul count vs bf16
        (the PE streams 1 col/cycle either way), and halves the DMA
        bytes again on top of bf16.  Per-tap LDWEIGHTS reloads inside a
        chunk loop are deleted post-schedule (identical stationary; see
        _dedup_ldweights) with a nosync MM chain pinning PE order.  PSUM
        accumulates in fp32; scalar/vector extract+scale to bf16.  Host
        pre-pads images (58-wide rows, zero borders, cin pair bytes
        interleaved) and pre-gathers the selected channels' weights into
        9 stationary [128cin x 2 x 128cout] DR slabs per unit, so the
        device does nothing but DMA + 9xN DR matmuls + extraction + DMA.
"""

import math
import os
import sys
import types

sys.path.insert(0, "/opt/trn_rl_repo")

import numpy as np
import ml_dtypes

BF16_NP = ml_dtypes.bfloat16

# antenv.axon_hooks is missing from this image; inject a minimal stand-in so
# run_bass_kernel_spmd's trace path imports cleanly (used only when tracing).
try:
    import antenv  # noqa: F401

    if "antenv.axon_hooks" not in sys.modules:
        _m = types.ModuleType("antenv.axon_hooks")
        _h = [None]
        _m.set_axon_ntff_profile_hook = lambda hook: _h.__setitem__(0, hook)
        _m.get_axon_ntff_profile_hook = lambda: _h[0]
        sys.modules["antenv.axon_hooks"] = _m
        antenv.axon_hooks = _m
except Exception:
    pass

import concourse.mybir as mybir
from concourse import bacc
from concourse.tile import TileContext
from concourse.bass_utils import run_bass_kernel_spmd

F32 = mybir.dt.float32
BF16 = mybir.dt.bfloat16
FP8 = mybir.dt.float8e4
AF = mybir.ActivationFunctionType
ALU = mybir.AluOpType
DR = mybir.MatmulPerfMode.DoubleRow
E4_NP = ml_dtypes.float8_e4m3

# fp8 DoubleRow split-conv: out = sum over PASSES (w_half . x_half) / WSCALE
# with Wh/Wl, Xh/Xl the fp8 hi/lo split of WSCALE*conv_w and x.  DoubleRow
# contracts both 128-channel blocks per instruction at 0.5 cycles/col.
# PASSES=((0,0),) is the 1-pass Wh.Xh variant: host-emulated end-to-end
# rel err 1.8172e-2 (deterministic; gate is 2e-2).  ((0,0),(1,0)) would be
# the 1.29e-2 2-pass fallback at twice the PE time.
USE_FP8 = True
PASSES = ((0, 0),)
NEED_WL = any(w == 1 for w, _ in PASSES)
NEED_XL = any(x == 1 for _, x in PASSES)
# Emit LDWEIGHTS once per tap group instead of per matmul (the k-loop
# reuses the stationary): after Tile scheduling, delete InstLdweights whose
# stationary AP is identical to the previous PE weight load; a nosync dep
# chain pins the PE order the surgery relies on.
DEDUP_LDW = True
WSCALE = 64.0


def _dedup_ldweights(nc):
    """Remove back-to-back-identical InstLdweights (per-matmul reloads of
    the same stationary).  The PE keeps the loaded stationary across
    matmuls; walrus pairs each matmul with the most recent load."""
    removed = 0
    for bb in nc.main_func.blocks:
        prev_key = None
        for i in list(bb.instructions):
            if isinstance(i, mybir.InstLdweights):
                key = (str(i.ins[0]), str(i.tile_size), str(i.tile_position),
                       str(i.perf_mode), str(i.is_transpose))
                if key == prev_key and not i.sync_info:
                    bb.instructions.remove(i)
                    removed += 1
                    continue
                prev_key = key
            elif isinstance(i, mybir.InstMatmult):
                pass                     # matmul keeps the stationary loaded
            elif i.engine == mybir.EngineType.PE:
                prev_key = None          # other PE ops: be conservative
    return removed

B, C, H, W = 32, 256, 56, 56
LSTM_H = 10
NCORES = 8
PH, PW = H + 2, W + 2          # 58x58 zero-padded image
NCHUNK = 7                     # 7 chunks x 8 output rows = 56
CH_ROWS = 8
CH_N = CH_ROWS * PW            # 464 moving cols per chunk matmul
TAIL = 4                       # tap (2,2) of the last chunk reads 2 past the end

_CACHE = {}


# ---------------------------------------------------------------- host gates

def _sigmoid(z):
    return 1.0 / (1.0 + np.exp(-z))


def _host_gates(inputs):
    """Exact gate replication.  Returns {sample: sel_channel_idx_array}."""
    x = np.asarray(inputs["x"], np.float32)

    # layer gate: GAP -> 1x1 conv -> single-step LSTM from zero state -> fc
    g = x.mean(axis=(2, 3), dtype=np.float64)                      # (B, C)
    lgw = np.asarray(inputs["lg_conv_w"], np.float64).reshape(LSTM_H, C)
    h = np.maximum(g @ lgw.T + np.asarray(inputs["lg_conv_b"], np.float64), 0.0)
    gates = (h @ np.asarray(inputs["lstm_w_ih"], np.float64).T
             + np.asarray(inputs["lstm_b_ih"], np.float64)
             + np.asarray(inputs["lstm_b_hh"], np.float64))
    i_, f_, g_, o_ = np.split(gates, 4, axis=1)
    c = _sigmoid(i_) * np.tanh(g_)
    hs = _sigmoid(o_) * np.tanh(c)
    lpre = hs @ np.asarray(inputs["lg_fc_w"], np.float64).T \
        + np.asarray(inputs["lg_fc_b"], np.float64)
    # round(sigmoid(relu(z))) == 1  iff  z > 0   (round-half-even at z == 0)
    layer_on = lpre[:, 0] > 0.0

    # channel gate (only for layer-active samples): s2 valid 3x3 conv -> relu
    # -> GAP -> fc; mask_c = (fc_pre > 0)
    cg_w = np.asarray(inputs["cg_conv_w"], np.float32)
    cg_b = np.asarray(inputs["cg_conv_b"], np.float32)
    fc_w = np.asarray(inputs["cg_fc_w"], np.float64)
    fc_b = np.asarray(inputs["cg_fc_b"], np.float64)
    W2 = cg_w.reshape(C, C * 9)                    # [o, c*9 + dy*3 + dx]

    sel = {}
    for b in np.where(layer_on)[0]:
        cols = np.empty((C, 9, 27, 27), np.float32)
        for tap in range(9):
            dy, dx = tap // 3, tap % 3
            cols[:, tap] = x[b][:, dy:dy + 53:2, dx:dx + 53:2]
        pre = W2 @ cols.reshape(C * 9, 27 * 27)    # (C, 729)
        hrel = np.maximum(pre + cg_b[:, None], 0.0)
        gap = hrel.mean(axis=1, dtype=np.float64)  # (C,)
        f = fc_w @ gap + fc_b
        mask = f > 0.0
        if mask.any():
            sel[int(b)] = np.where(mask)[0]
    return sel


# ---------------------------------------------------------------- scheduling

def _schedule(sel):
    """Pack conv work into an identical per-core slot template.

    Units: (sample, <=128 selected channels).  Each unit is 7 chunks of 8
    output rows.  Template [m_0 >= m_1 >= ...] identical on every core
    (SPMD); pieces of a unit are contiguous chunk ranges placed into slots.

    Returns (template, assign) where assign[core][slot] is either None or
    (b, sel_ids, a0, r0, r1): slot computes chunks [a0, a0+m) of sample b,
    of which [r0, r1) are used for output.
    """
    units = []
    for b, ids in sorted(sel.items()):
        for lo in range(0, len(ids), 128):
            units.append((b, ids[lo:lo + 128]))
    n = len(units)
    if n == 0:
        return [1], [[None] for _ in range(NCORES)]

    q = math.ceil(NCHUNK * n / NCORES)
    while True:
        template = [NCHUNK] * (q // NCHUNK)
        r = q % NCHUNK
        if r:
            template.append(r)
        n7 = NCORES * (q // NCHUNK)
        whole = min(n, n7)
        leftover = units[whole:]
        # leftover units are split into ceil(7/r) pieces of size r each,
        # all placed in the r-slots (NCORES available)
        if leftover and (not r or len(leftover) * math.ceil(NCHUNK / r) > NCORES):
            q += 1
            continue
        break

    assign = [[None] * len(template) for _ in range(NCORES)]
    # whole units -> 7-slots, round robin
    for i in range(whole):
        core = i % NCORES
        slot = i // NCORES
        b, ids = units[i]
        assign[core][slot] = (b, ids, 0, 0, NCHUNK)
    # leftover units -> r-slots, pieces of exactly r chunks
    rslot = len(template) - 1
    core = 0
    for b, ids in leftover:
        r0 = 0
        while r0 < NCHUNK:
            r1 = min(r0 + template[rslot], NCHUNK)
            a0 = min(r0, NCHUNK - template[rslot])   # shift window if short
            assign[core][rslot] = (b, ids, a0, r0, r1)
            core += 1
            r0 = r1
    return template, assign


# ---------------------------------------------------------------- device

def _ccols(m):
    # 16-byte-align the DR moving-pair stride (cols): the DoubleRow AP's
    # inter-row step must be a multiple of 16
    c = (8 * m + 2) * PW + TAIL
    return (c + 15) // 16 * 16


def _build_fp8(template):
    from concourse.tile_rust import add_dep_helper

    nc = bacc.Bacc(None, target_bir_lowering=False)
    NW = 2 if NEED_WL else 1
    NX = 2 if NEED_XL else 1
    WN, XN = ("wh", "wl"), ("xh", "xl")

    wds, xds, outds = [], [], []
    for s, m in enumerate(template):
        cols = _ccols(m)
        wds.append([nc.declare_dram_parameter(
            f"{WN[i]}{s}", [128, 9 * 256], FP8, isOutput=False)
            for i in range(NW)])
        xds.append([nc.declare_dram_parameter(
            f"{XN[i]}{s}", [128, 2 * cols], FP8, isOutput=False)
            for i in range(NX)])
        outds.append(nc.declare_dram_parameter(
            f"outd{s}", [128, m * CH_ROWS * W], BF16, isOutput=True))

    with TileContext(nc) as tc:
        with tc.tile_pool(name="work", bufs=1) as pw, \
             tc.tile_pool(name="psum", bufs=1, space="PSUM") as pp:

            wts = [[pw.tile([128, 9 * 256], FP8, tag=f"{WN[i]}{s}",
                            name=f"{WN[i]}{s}")
                    for i in range(NW)] for s in range(len(template))]
            xts = [[pw.tile([128, 2 * _ccols(m)], FP8, tag=f"{XN[i]}{s}",
                            name=f"{XN[i]}{s}")
                    for i in range(NX)] for s, m in enumerate(template)]

            # DoubleRow views: stationary [128, tap, 2, 128cout], moving
            # [128, 2, cols].  The moving pair (cin c, c+128) is interleaved
            # byte-adjacent in memory (pair stride 1, col stride 2) so the
            # PE can fetch a full fp8 pair per 2-byte read; with the two
            # halves in separate far-apart streams it fills at 1 cyc/col
            # instead of 0.5.
            wvs = [[w[:].rearrange("p (t two j) -> p t two j", t=9, two=2)
                    for w in ws] for ws in wts]
            xvs = [[x[:].rearrange("p (n two) -> p two n", two=2)
                    for x in xs] for xs in xts]

            # DMA emission split across the two HWDGE rings (sync=qSP,
            # scalar=qAct — transfers are FIFO per ring, ~2us completion
            # latency each), in criticality order.  The first matmul needs
            # only wh0 tap0 (64KB) + the first xh0 row piece; wh0's other
            # taps ride the scalar ring in parallel.
            m0 = template[0]
            cols0 = _ccols(m0)
            # slot-0 weights ride the scalar ring as ONE transfer (per-DMA
            # completion receipts are ~2.4us and serialize per ring) so the
            # sync ring's receipts belong to the x pieces alone
            nc.scalar.dma_start(out=wts[0][0][:], in_=wds[0][0][:])
            # row-piece boundaries matched to the slot-0 compute phases
            # (chunks 0-1 read rows 0-17, 2-3 rows 16-33, 4-6 rows 32-57)
            rb = [0, 18, 34] if m0 == NCHUNK else [0]
            pieces = [r * PW for r in rb] + [cols0]
            # interleaved layout: piece [c0, c1) in cols = contiguous bytes
            for c0, c1 in zip(pieces, pieces[1:]):
                nc.sync.dma_start(out=xts[0][0][:, 2 * c0:2 * c1],
                                  in_=xds[0][0][:, 2 * c0:2 * c1])
            for i in range(1, NW):
                nc.scalar.dma_start(out=wts[0][i][:], in_=wds[0][i][:])
            for i in range(1, NX):
                nc.scalar.dma_start(out=xts[0][i][:], in_=xds[0][i][:])
            # later slots' inputs are deferred until the first matmul has
            # run: at t=0 all 8 cores burst their full input set at HBM
            # and the critical slot-0 completion receipts (which gate the
            # first matmul) pay the high-load latency.  Emitted after the
            # slot-0 compute with a sync dep on the first matmul.
            deferred = []
            for s in range(1, len(template)):
                ring = nc.scalar if s % 2 == 1 else nc.sync
                for i in range(NW):
                    deferred.append((ring, wts[s][i], wds[s][i]))
                for i in range(NX):
                    deferred.append((ring, xts[s][i], xds[s][i]))

            prev_mm = [None]
            anchor = [None]

            def mm_chain(inst):
                if DEDUP_LDW:
                    if prev_mm[0] is not None:
                        add_dep_helper(inst.ins, prev_mm[0].ins, sync=False,
                                       reason="pe-order for ldweights dedup")
                    prev_mm[0] = inst

            # warm the PE (p-state ramp) while slot-0 data lands; memset-fed
            # so it starts as soon as the engines come alive, sized to cover
            # the first-piece DMA latency (~1.5us)
            wsrc = pw.tile([128, 512], FP8, tag="wsrc")
            nc.vector.memset(wsrc[:], 0.0)
            wsv = wsrc[:].rearrange("p (two j) -> p two j", two=2)
            wps = pp.tile([128, 256], F32, tag="warmps")
            for _ in range(16):
                w = nc.tensor.matmul(wps[:], wsv[:, :, 0:128], wsv[:],
                                     start=True, stop=True, perf_mode=DR,
                                     skip_group_check=True)
                mm_chain(w)

            npass = len(PASSES)
            for s, m in enumerate(template):
                banks = [pp.tile([128, CH_N], F32, tag=f"bank{k}",
                                 name=f"bank{s}_{k}")
                         for k in range(m)]
                # slot 0 runs in chunk-range phases matched to the xh piece
                # boundaries, so compute never outruns the wire
                if s == 0 and m == NCHUNK:
                    kranges = [(0, 2), (2, 4), (4, m)]
                else:
                    kranges = [(0, m)]
                for klo, khi in kranges:
                    for g in range(9 * npass):
                        p, tap = divmod(g, 9)
                        wsel, xsel = PASSES[p]
                        dy, dx = tap // 3, tap % 3
                        for k in range(klo, khi):
                            off = (CH_ROWS * k + dy) * PW + dx
                            w = nc.tensor.matmul(
                                banks[k][:, 0:CH_N - 2],
                                wvs[s][wsel][:, tap],
                                xvs[s][xsel][:, :, off:off + CH_N - 2],
                                start=(g == 0), stop=(g == 9 * npass - 1),
                                perf_mode=DR, skip_group_check=True)
                            mm_chain(w)
                            if anchor[0] is None:
                                anchor[0] = w
                if s == 0:
                    for ring, o, i_ in deferred:
                        dd = ring.dma_start(out=o[:], in_=i_[:])
                        add_dep_helper(dd.ins, anchor[0].ins, sync=True,
                                       reason="defer noncritical input DMA "
                                              "past the first matmul")

                stg = pw.tile([128, m * CH_ROWS * W], BF16, tag=f"stg{s}")
                sv = stg[:].rearrange("p (r c) -> p r c", c=W)
                for k in range(m):
                    bv = banks[k][:].rearrange("p (r c) -> p r c", c=PW)
                    if m == 1:
                        # program-tail chunk: split across both engines
                        half = CH_ROWS // 2
                        nc.scalar.activation(
                            sv[:, 0:half, :], bv[:, 0:half, 0:W],
                            AF.Copy, scale=1.0 / WSCALE)
                        nc.vector.tensor_scalar(
                            out=sv[:, half:CH_ROWS, :],
                            in0=bv[:, half:CH_ROWS, 0:W],
                            scalar1=1.0 / WSCALE, scalar2=None, op0=ALU.mult)
                    elif k % 2 == 0:
                        nc.scalar.activation(
                            sv[:, k * CH_ROWS:(k + 1) * CH_ROWS, :],
                            bv[:, :, 0:W], AF.Copy, scale=1.0 / WSCALE)
                    else:
                        nc.vector.tensor_scalar(
                            out=sv[:, k * CH_ROWS:(k + 1) * CH_ROWS, :],
                            in0=bv[:, :, 0:W],
                            scalar1=1.0 / WSCALE, scalar2=None, op0=ALU.mult)
                # gpsimd SWDGE ring (otherwise idle) for mid-program
                # outputs; two pieces so the first chunks' rows stream out
                # while the tail chunks are still extracting.  The last
                # slot's output takes the (by then idle) sync HWDGE ring:
                # lower first-byte latency on the program's critical tail.
                last = s == len(template) - 1
                ring = nc.sync if last else nc.gpsimd
                if m > 4:
                    cmid = 4 * CH_ROWS * W
                    ring.dma_start(out=outds[s][:, 0:cmid],
                                   in_=stg[:, 0:cmid])
                    ring.dma_start(out=outds[s][:, cmid:],
                                   in_=stg[:, cmid:])
                else:
                    ring.dma_start(out=outds[s][:], in_=stg[:])

    if DEDUP_LDW:
        _dedup_ldweights(nc)
    nc.compile()
    return nc


def _build(template):
    if USE_FP8:
        return _build_fp8(template)
    nc = bacc.Bacc(None, target_bir_lowering=False)

    xins, wsls, outds = [], [], []
    for s, m in enumerate(template):
        cols = (8 * m + 2) * PW + TAIL
        xins.append([nc.declare_dram_parameter(
            f"xin{s}_{kb}", [128, cols], BF16, isOutput=False)
            for kb in range(2)])
        wsls.append([nc.declare_dram_parameter(
            f"wsl{s}_{kb}", [128, 9 * 128], BF16, isOutput=False)
            for kb in range(2)])
        outds.append(nc.declare_dram_parameter(
            f"outd{s}", [128, m * CH_ROWS * W], BF16, isOutput=True))

    with TileContext(nc) as tc:
        with tc.tile_pool(name="work", bufs=1) as pw, \
             tc.tile_pool(name="psum", bufs=1, space="PSUM") as pp:

            wts, xbs = [], []
            for s, m in enumerate(template):
                cols = (8 * m + 2) * PW + TAIL
                wts.append([pw.tile([128, 9 * 128], BF16, tag=f"w{s}_{kb}",
                                    name=f"w{s}_{kb}")
                            for kb in range(2)])
                xbs.append([pw.tile([128, cols], BF16, tag=f"x{s}_{kb}",
                                    name=f"x{s}_{kb}")
                            for kb in range(2)])

            # DMA emission in criticality order: queues drain descriptors in
            # instruction order, so slot 0's kb=0 data (needed by the first
            # 9 matmul groups) comes first.  xin0_0 is split into row-pieces:
            # group 0's k-loop walks rows bottom-up, and range-level hazard
            # tracking lets chunk k's matmul start when its piece lands.
            m0 = template[0]
            nrow0 = 8 * m0 + 2
            # boundaries aligned to the two slot-0 compute phases
            # (chunks 0-3 read rows 0-33, chunks 4-6 read rows 32-57)
            if m0 == NCHUNK:
                rb = [0, 18, 34, nrow0]
            else:
                rb = [0, nrow0]
            pieces = [r * PW for r in rb[:-1]] + [nrow0 * PW + TAIL]
            # only wsl0_0's tap-0 slice (32 KB) gates the first matmul; the
            # other taps are consumed over the next 12 us, so they stream
            # behind the image whose completion bounds the kb0 phase
            nc.sync.dma_start(out=wts[0][0][:, 0:128], in_=wsls[0][0][:, 0:128])
            for c0, c1 in zip(pieces, pieces[1:]):
                nc.sync.dma_start(out=xbs[0][0][:, c0:c1],
                                  in_=xins[0][0][:, c0:c1])
            nc.sync.dma_start(out=wts[0][0][:, 128:], in_=wsls[0][0][:, 128:])
            nc.sync.dma_start(out=wts[0][1][:], in_=wsls[0][1][:])
            nc.sync.dma_start(out=xbs[0][1][:], in_=xins[0][1][:])
            for s in range(1, len(template)):
                for kb in range(2):
                    nc.sync.dma_start(out=wts[s][kb][:], in_=wsls[s][kb][:])
                    nc.sync.dma_start(out=xbs[s][kb][:], in_=xins[s][kb][:])

            # warm the PE (p-state ramp) while slot-0 data lands; memset-fed
            # so the warm-up starts as soon as the engines come alive, and
            # long enough (~6us) that the PE does not idle-reset its ramp
            # before the first conv matmul's data arrives
            wsrc = pw.tile([128, 256], BF16, tag="wsrc")
            nc.vector.memset(wsrc[:], 0.0)
            wps = pp.tile([128, 256], F32, tag="warmps")
            for _ in range(24):
                nc.tensor.matmul(wps[:, 0:128], wsrc[:, 0:128], wsrc[:, 0:128],
                                 start=True, stop=True, skip_group_check=True)
            for _ in range(6):
                nc.tensor.matmul(wps[:], wsrc[:, 0:128], wsrc[:],
                                 start=True, stop=True, skip_group_check=True)

            for s, m in enumerate(template):
                banks = [pp.tile([128, CH_N], F32, tag=f"bank{k}",
                                 name=f"bank{s}_{k}")
                         for k in range(m)]
                # slot 0 kb0 runs in two chunk-range phases matched to the
                # xin piece boundaries, so compute never outruns the wire
                if s == 0 and m == NCHUNK:
                    kranges = [(0, 4), (4, NCHUNK)]
                else:
                    kranges = [(0, m)]
                for klo, khi in kranges:
                    for g in range(9):
                        tap = g
                        dy, dx = tap // 3, tap % 3
                        for k in range(klo, khi):
                            off = (CH_ROWS * k + dy) * PW + dx
                            nc.tensor.matmul(
                                banks[k][:, 0:CH_N - 2],
                                wts[s][0][:, tap * 128:(tap + 1) * 128],
                                xbs[s][0][:, off:off + CH_N - 2],
                                start=(g == 0), stop=False,
                                skip_group_check=True)
                for g in range(9, 18):
                    tap = g - 9
                    dy, dx = tap // 3, tap % 3
                    for k in range(m):
                        off = (CH_ROWS * k + dy) * PW + dx
                        nc.tensor.matmul(
                            banks[k][:, 0:CH_N - 2],
                            wts[s][1][:, tap * 128:(tap + 1) * 128],
                            xbs[s][1][:, off:off + CH_N - 2],
                            start=False, stop=(g == 17),
                            skip_group_check=True)

                stg = pw.tile([128, m * CH_ROWS * W], BF16, tag=f"stg{s}")
                sv = stg[:].rearrange("p (r c) -> p r c", c=W)
                for k in range(m):
                    bv = banks[k][:].rearrange("p (r c) -> p r c", c=PW)
                    if k % 2 == 0:
                        nc.scalar.activation(
                            sv[:, k * CH_ROWS:(k + 1) * CH_ROWS, :],
                            bv[:, :, 0:W], AF.Copy)
                    else:
                        nc.vector.tensor_copy(
                            out=sv[:, k * CH_ROWS:(k + 1) * CH_ROWS, :],
                            in_=bv[:, :, 0:W])
                # two pieces so the first chunks' rows stream out while the
                # tail chunks are still being extracted; sync ring is idle
                # by output time (scalar stays copy-only)
                if m > 4:
                    cmid = 4 * CH_ROWS * W
                    nc.sync.dma_start(out=outds[s][:, 0:cmid],
                                      in_=stg[:, 0:cmid])
                    nc.sync.dma_start(out=outds[s][:, cmid:],
                                      in_=stg[:, cmid:])
                else:
                    nc.sync.dma_start(out=outds[s][:], in_=stg[:])

    nc.compile()
    return nc


# ---------------------------------------------------------------- packing

def _pack_inputs_fp8(inputs, template, assign):
    x = np.asarray(inputs["x"], np.float32)
    conv_w = np.asarray(inputs["conv_w"], np.float32)

    # per-sample padded fp8 hi/lo image pair, built lazily
    padded = {}

    def pimg(b):
        if b not in padded:
            p = np.zeros((C, PH, PW), np.float32)
            p[:, 1:57, 1:57] = x[b]
            hi = p.astype(E4_NP)
            lo = (p - hi.astype(np.float32)).astype(E4_NP) if NEED_XL else None
            padded[b] = (hi, lo)
        return padded[b]

    # per-unit hi/lo slabs: slab[p, tap*256 + kb*128 + j] =
    # WSCALE*conv_w[sel_j, kb*128 + p, dy, dx], fp8-split
    slabs = {}

    def slab(b, ids):
        key = (b, ids.tobytes())
        if key not in slabs:
            sl = np.zeros((128, 9 * 256), np.float32)
            wsel = conv_w[ids] * WSCALE              # [n, C, 3, 3]
            n = len(ids)
            for tap in range(9):
                dy, dx = tap // 3, tap % 3
                for kb in range(2):
                    col = tap * 256 + kb * 128
                    sl[:, col:col + n] = \
                        wsel[:, kb * 128:(kb + 1) * 128, dy, dx].T
            hi = sl.astype(E4_NP)
            lo = (sl - hi.astype(np.float32)).astype(E4_NP) if NEED_WL else None
            slabs[key] = (hi, lo)
        return slabs[key]

    in_maps = []
    for core in range(NCORES):
        m_map = {}
        for s, m in enumerate(template):
            cols = _ccols(m)
            xh = np.zeros((128, 2 * cols), E4_NP)
            xl = np.zeros((128, 2 * cols), E4_NP) if NEED_XL else None
            wh = np.zeros((128, 9 * 256), E4_NP)
            wl = np.zeros((128, 9 * 256), E4_NP) if NEED_WL else None
            a = assign[core][s]
            if a is not None:
                b, ids, a0, _, _ = a
                hi, lo = pimg(b)
                nrow = 8 * m + 2
                rh = hi[:, 8 * a0:8 * a0 + nrow, :].reshape(C, -1)
                nr = rh.shape[1]
                # interleave the cin pair (c, c+128) byte-adjacent:
                # xh[p, 2*i + kb] = x[kb*128 + p, i]
                xhv = xh.reshape(128, cols, 2)
                for kb in range(2):
                    xhv[:, :nr, kb] = rh[kb * 128:(kb + 1) * 128]
                if NEED_XL:
                    rl = lo[:, 8 * a0:8 * a0 + nrow, :].reshape(C, -1)
                    xlv = xl.reshape(128, cols, 2)
                    for kb in range(2):
                        xlv[:, :nr, kb] = rl[kb * 128:(kb + 1) * 128]
                whs, wls = slab(b, ids)
                wh[:] = whs
                if NEED_WL:
                    wl[:] = wls
            m_map[f"xh{s}"] = xh
            m_map[f"wh{s}"] = wh
            if NEED_XL:
                m_map[f"xl{s}"] = xl
            if NEED_WL:
                m_map[f"wl{s}"] = wl
        in_maps.append(m_map)
    return in_maps


def _pack_inputs(inputs, template, assign):
    if USE_FP8:
        return _pack_inputs_fp8(inputs, template, assign)
    x = np.asarray(inputs["x"], np.float32)
    conv_w = np.asarray(inputs["conv_w"], np.float32)

    # per-sample padded bf16 image, built lazily
    padded = {}

    def pimg(b):
        if b not in padded:
            p = np.zeros((C, PH, PW), np.float32)
            p[:, 1:57, 1:57] = x[b]
            padded[b] = p.astype(BF16_NP)
        return padded[b]

    # per-unit weight slabs, built lazily:  slab[kb][cin, tap*128 + i] =
    # conv_w[sel_i, kb*128 + cin, dy, dx]
    slabs = {}

    def slab(b, ids):
        key = (b, ids.tobytes())
        if key not in slabs:
            sl = np.zeros((2, 128, 9 * 128), np.float32)
            wsel = conv_w[ids]                       # [n, C, 3, 3]
            n = len(ids)
            for tap in range(9):
                dy, dx = tap // 3, tap % 3
                for kb in range(2):
                    sl[kb, :, tap * 128:tap * 128 + n] = \
                        wsel[:, kb * 128:(kb + 1) * 128, dy, dx].T
            slabs[key] = sl.astype(BF16_NP)
        return slabs[key]

    in_maps = []
    for core in range(NCORES):
        m_map = {}
        for s, m in enumerate(template):
            cols = (8 * m + 2) * PW + TAIL
            xin = np.zeros((2, 128, cols), BF16_NP)
            wsl = np.zeros((2, 128, 9 * 128), BF16_NP)
            a = assign[core][s]
            if a is not None:
                b, ids, a0, _, _ = a
                rows = pimg(b)[:, 8 * a0:8 * a0 + 8 * m + 2, :] \
                    .reshape(C, -1)                  # [C, (8m+2)*58]
                xin[0, :, :rows.shape[1]] = rows[:128]
                xin[1, :, :rows.shape[1]] = rows[128:]
                wsl[:] = slab(b, ids)
            for kb in range(2):
                m_map[f"xin{s}_{kb}"] = xin[kb]
                m_map[f"wsl{s}_{kb}"] = wsl[kb]
        in_maps.append(m_map)
    return in_maps


def _assemble(inputs, template, assign, results):
    x = np.asarray(inputs["x"], np.float32)
    out = x.copy()
    for core in range(NCORES):
        for s, m in enumerate(template):
            a = assign[core][s]
            if a is None:
                continue
            b, ids, a0, r0, r1 = a
            n = len(ids)
            data = np.asarray(results[core][f"outd{s}"]) \
                .reshape(128, m * CH_ROWS, W)[:n].astype(np.float32)
            lk0, lk1 = r0 - a0, r1 - a0
            out[b, ids, 8 * r0:8 * r1, :] = \
                data[:, lk0 * CH_ROWS:lk1 * CH_ROWS, :]
    return out


# ---------------------------------------------------------------- entry

def kernel(**inputs):
    sel = _host_gates(inputs)
    template, assign = _schedule(sel)

    tkey = tuple(template)
    if _CACHE.get("tkey") != tkey:
        _CACHE["nc"] = _build(template)
        _CACHE["tkey"] = tkey
    nc = _CACHE["nc"]

    in_maps = _pack_inputs(inputs, template, assign)

    trace = bool(int(os.environ.get("BASS_KERNEL_TRACE", "0")))
    kw = {}
    if trace:
        from trn_agent_boot.trn_boot import _ntff_profile_via_ctypes
        import antenv.axon_hooks as ah
        ah.set_axon_ntff_profile_hook(
            _ntff_profile_via_ctypes("/opt/axon/libaxon_pjrt.so"))
        import tempfile
        base = os.environ.get("BASS_KERNEL_TRACE_DIR", "/tmp/adaptconv_trace")
        os.makedirs(base, exist_ok=True)
        kw = dict(trace=True, tmpdir=tempfile.mkdtemp(dir=base))

    res = run_bass_kernel_spmd(nc, in_maps, core_ids=list(range(NCORES)), **kw)
    _CACHE["last_exec_time_ns"] = res.exec_time_ns

    return _assemble(inputs, template, assign, res.results)



# revision 41
# speedup vs baseline: 1.0262x; 1.0262x over previous
"""AdaptConv2d Trainium2 kernel: host-routed, balanced 8-core sparse conv.

The gates (layer LSTM gate + channel gate) are tiny compared to the main
conv, but they are data-dependent and the active samples cluster badly
under a contiguous batch split (SPMD time = slowest core).  So:

  Host: computes both gates exactly in fp64-tailed numpy (margins on the
        binary decisions are ~1e-3; fp32/fp64 host math is ~1e-6 off the
        fp32 jax reference, so decisions match).  Pass-through channels
        (out = x) are assembled on host.  Only the ~17 active samples'
        ~116 selected channels need conv on device.

  Device: a fully static SPMD program - no If/For_i/values_load/indirect
        DMA.  Work is chunked at (sample, 8-output-row) granularity and
        packed into an identical per-core slot template (e.g. [7,7,1] =
        15 chunks/core for 119 total chunks), so all 8 cores finish
        together.  The conv runs as 1-pass fp8e4m3 DoubleRow matmuls
        (Wh.Xh with WSCALE*w and x round-to-nearest fp8; end-to-end rel
        err 1.817e-2 vs the 2e-2 gate, deterministic): DR contracts both
        128-cin blocks per instruction, halving the matmul count vs bf16
        (the PE streams 1 col/cycle either way), and halves the DMA
        bytes again on top of bf16.  Per-tap LDWEIGHTS reloads inside a
        chunk loop are deleted post-schedule (identical stationary; see
        _dedup_ldweights) with a nosync MM chain pinning PE order.  PSUM
        accumulates in fp32; scalar/vector extract+scale to bf16.  Host
        pre-pads images (58-wide rows, zero borders, cin pair bytes
        interleaved) and pre-gathers the selected channels' weights into
        9 stationary [128cin x 2 x 128cout] DR slabs per unit, so the
        device does nothing but DMA + 9xN DR matmuls + extraction + DMA.
"""

import math
import os
import sys
import types

sys.path.insert(0, "/opt/trn_rl_repo")

import numpy as np
import ml_dtypes

BF16_NP = ml_dtypes.bfloat16

# antenv.axon_hooks is missing from this image; inject a minimal stand-in so
# run_bass_kernel_spmd's trace path imports cleanly (used only when tracing).
try:
    import antenv  # noqa: F401

    if "antenv.axon_hooks" not in sys.modules:
        _m = types.ModuleType("antenv.axon_hooks")
        _h = [None]
        _m.set_axon_ntff_profile_hook = lambda hook: _h.__setitem__(0, hook)
        _m.get_axon_ntff_profile_hook = lambda: _h[0]
        sys.modules["antenv.axon_hooks"] = _m
        antenv.axon_hooks = _m
except Exception:
    pass

import concourse.mybir as mybir
from concourse import bacc
from concourse.tile import TileContext
from concourse.bass_utils import run_bass_kernel_spmd

F32 = mybir.dt.float32
BF16 = mybir.dt.bfloat16
FP8 = mybir.dt.float8e4
AF = mybir.ActivationFunctionType
ALU = mybir.AluOpType
DR = mybir.MatmulPerfMode.DoubleRow
E4_NP = ml_dtypes.float8_e4m3

# fp8 DoubleRow split-conv: out = sum over PASSES (w_half . x_half) / WSCALE
# with Wh/Wl, Xh/Xl the fp8 hi/lo split of WSCALE*conv_w and x.  DoubleRow
# contracts both 128-channel blocks per instruction at 0.5 cycles/col.
# PASSES=((0,0),) is the 1-pass Wh.Xh variant: host-emulated end-to-end
# rel err 1.8172e-2 (deterministic; gate is 2e-2).  ((0,0),(1,0)) would be
# the 1.29e-2 2-pass fallback at twice the PE time.
USE_FP8 = True
PASSES = ((0, 0),)
NEED_WL = any(w == 1 for w, _ in PASSES)
NEED_XL = any(x == 1 for _, x in PASSES)
# Emit LDWEIGHTS once per tap group instead of per matmul (the k-loop
# reuses the stationary): after Tile scheduling, delete InstLdweights whose
# stationary AP is identical to the previous PE weight load; a nosync dep
# chain pins the PE order the surgery relies on.
DEDUP_LDW = True
WSCALE = 64.0


def _dedup_ldweights(nc):
    """Remove back-to-back-identical InstLdweights (per-matmul reloads of
    the same stationary).  The PE keeps the loaded stationary across
    matmuls; walrus pairs each matmul with the most recent load."""
    removed = 0
    for bb in nc.main_func.blocks:
        prev_key = None
        for i in list(bb.instructions):
            if isinstance(i, mybir.InstLdweights):
                key = (str(i.ins[0]), str(i.tile_size), str(i.tile_position),
                       str(i.perf_mode), str(i.is_transpose))
                if key == prev_key and not i.sync_info:
                    bb.instructions.remove(i)
                    removed += 1
                    continue
                prev_key = key
            elif isinstance(i, mybir.InstMatmult):
                pass                     # matmul keeps the stationary loaded
            elif i.engine == mybir.EngineType.PE:
                prev_key = None          # other PE ops: be conservative
    return removed

B, C, H, W = 32, 256, 56, 56
LSTM_H = 10
NCORES = 8
PH, PW = H + 2, W + 2          # 58x58 zero-padded image
NCHUNK = 7                     # 7 chunks x 8 output rows = 56
CH_ROWS = 8
CH_N = CH_ROWS * PW            # 464 moving cols per chunk matmul
TAIL = 4                       # tap (2,2) of the last chunk reads 2 past the end

_CACHE = {}


# ---------------------------------------------------------------- host gates

def _sigmoid(z):
    return 1.0 / (1.0 + np.exp(-z))


def _host_gates(inputs):
    """Exact gate replication.  Returns {sample: sel_channel_idx_array}."""
    x = np.asarray(inputs["x"], np.float32)

    # layer gate: GAP -> 1x1 conv -> single-step LSTM from zero state -> fc
    g = x.mean(axis=(2, 3), dtype=np.float64)                      # (B, C)
    lgw = np.asarray(inputs["lg_conv_w"], np.float64).reshape(LSTM_H, C)
    h = np.maximum(g @ lgw.T + np.asarray(inputs["lg_conv_b"], np.float64), 0.0)
    gates = (h @ np.asarray(inputs["lstm_w_ih"], np.float64).T
             + np.asarray(inputs["lstm_b_ih"], np.float64)
             + np.asarray(inputs["lstm_b_hh"], np.float64))
    i_, f_, g_, o_ = np.split(gates, 4, axis=1)
    c = _sigmoid(i_) * np.tanh(g_)
    hs = _sigmoid(o_) * np.tanh(c)
    lpre = hs @ np.asarray(inputs["lg_fc_w"], np.float64).T \
        + np.asarray(inputs["lg_fc_b"], np.float64)
    # round(sigmoid(relu(z))) == 1  iff  z > 0   (round-half-even at z == 0)
    layer_on = lpre[:, 0] > 0.0

    # channel gate (only for layer-active samples): s2 valid 3x3 conv -> relu
    # -> GAP -> fc; mask_c = (fc_pre > 0)
    cg_w = np.asarray(inputs["cg_conv_w"], np.float32)
    cg_b = np.asarray(inputs["cg_conv_b"], np.float32)
    fc_w = np.asarray(inputs["cg_fc_w"], np.float64)
    fc_b = np.asarray(inputs["cg_fc_b"], np.float64)
    W2 = cg_w.reshape(C, C * 9)                    # [o, c*9 + dy*3 + dx]

    sel = {}
    for b in np.where(layer_on)[0]:
        cols = np.empty((C, 9, 27, 27), np.float32)
        for tap in range(9):
            dy, dx = tap // 3, tap % 3
            cols[:, tap] = x[b][:, dy:dy + 53:2, dx:dx + 53:2]
        pre = W2 @ cols.reshape(C * 9, 27 * 27)    # (C, 729)
        hrel = np.maximum(pre + cg_b[:, None], 0.0)
        gap = hrel.mean(axis=1, dtype=np.float64)  # (C,)
        f = fc_w @ gap + fc_b
        mask = f > 0.0
        if mask.any():
            sel[int(b)] = np.where(mask)[0]
    return sel


# ---------------------------------------------------------------- scheduling

def _schedule(sel):
    """Pack conv work into an identical per-core slot template.

    Units: (sample, <=128 selected channels).  Each unit is 7 chunks of 8
    output rows.  Template [m_0 >= m_1 >= ...] identical on every core
    (SPMD); pieces of a unit are contiguous chunk ranges placed into slots.

    Returns (template, assign) where assign[core][slot] is either None or
    (b, sel_ids, a0, r0, r1): slot computes chunks [a0, a0+m) of sample b,
    of which [r0, r1) are used for output.
    """
    units = []
    for b, ids in sorted(sel.items()):
        for lo in range(0, len(ids), 128):
            units.append((b, ids[lo:lo + 128]))
    n = len(units)
    if n == 0:
        return [1], [[None] for _ in range(NCORES)]

    q = math.ceil(NCHUNK * n / NCORES)
    while True:
        template = [NCHUNK] * (q // NCHUNK)
        r = q % NCHUNK
        if r:
            template.append(r)
        n7 = NCORES * (q // NCHUNK)
        whole = min(n, n7)
        leftover = units[whole:]
        # leftover units are split into ceil(7/r) pieces of size r each,
        # all placed in the r-slots (NCORES available)
        if leftover and (not r or len(leftover) * math.ceil(NCHUNK / r) > NCORES):
            q += 1
            continue
        break

    assign = [[None] * len(template) for _ in range(NCORES)]
    # whole units -> 7-slots, round robin
    for i in range(whole):
        core = i % NCORES
        slot = i // NCORES
        b, ids = units[i]
        assign[core][slot] = (b, ids, 0, 0, NCHUNK)
    # leftover units -> r-slots, pieces of exactly r chunks
    rslot = len(template) - 1
    core = 0
    for b, ids in leftover:
        r0 = 0
        while r0 < NCHUNK:
            r1 = min(r0 + template[rslot], NCHUNK)
            a0 = min(r0, NCHUNK - template[rslot])   # shift window if short
            assign[core][rslot] = (b, ids, a0, r0, r1)
            core += 1
            r0 = r1
    return template, assign


# ---------------------------------------------------------------- device

def _ccols(m):
    # 16-byte-align the DR moving-pair stride (cols): the DoubleRow AP's
    # inter-row step must be a multiple of 16
    c = (8 * m + 2) * PW + TAIL
    return (c + 15) // 16 * 16


def _build_fp8(template):
    from concourse.tile_rust import add_dep_helper

    nc = bacc.Bacc(None, target_bir_lowering=False)
    NW = 2 if NEED_WL else 1
    NX = 2 if NEED_XL else 1
    WN, XN = ("wh", "wl"), ("xh", "xl")

    wds, xds, outds = [], [], []
    for s, m in enumerate(template):
        cols = _ccols(m)
        wds.append([nc.declare_dram_parameter(
            f"{WN[i]}{s}", [128, 9 * 256], FP8, isOutput=False)
            for i in range(NW)])
        xds.append([nc.declare_dram_parameter(
            f"{XN[i]}{s}", [128, 2 * cols], FP8, isOutput=False)
            for i in range(NX)])
        outds.append(nc.declare_dram_parameter(
            f"outd{s}", [128, m * CH_ROWS * W], BF16, isOutput=True))

    with TileContext(nc) as tc:
        with tc.tile_pool(name="work", bufs=1) as pw, \
             tc.tile_pool(name="psum", bufs=1, space="PSUM") as pp:

            wts = [[pw.tile([128, 9 * 256], FP8, tag=f"{WN[i]}{s}",
                            name=f"{WN[i]}{s}")
                    for i in range(NW)] for s in range(len(template))]
            xts = [[pw.tile([128, 2 * _ccols(m)], FP8, tag=f"{XN[i]}{s}",
                            name=f"{XN[i]}{s}")
                    for i in range(NX)] for s, m in enumerate(template)]

            # DoubleRow views: stationary [128, tap, 2, 128cout], moving
            # [128, 2, cols].  The moving pair (cin c, c+128) is interleaved
            # byte-adjacent in memory (pair stride 1, col stride 2) so the
            # PE can fetch a full fp8 pair per 2-byte read; with the two
            # halves in separate far-apart streams it fills at 1 cyc/col
            # instead of 0.5.
            wvs = [[w[:].rearrange("p (t two j) -> p t two j", t=9, two=2)
                    for w in ws] for ws in wts]
            xvs = [[x[:].rearrange("p (n two) -> p two n", two=2)
                    for x in xs] for xs in xts]

            # DMA emission split across the two HWDGE rings (sync=qSP,
            # scalar=qAct — transfers are FIFO per ring, ~2us completion
            # latency each), in criticality order.  The first matmul needs
            # only wh0 tap0 (64KB) + the first xh0 row piece; wh0's other
            # taps ride the scalar ring in parallel.
            m0 = template[0]
            cols0 = _ccols(m0)
            # slot-0 weights ride the scalar ring as ONE transfer (per-DMA
            # completion receipts are ~2.4us and serialize per ring) so the
            # sync ring's receipts belong to the x pieces alone
            nc.scalar.dma_start(out=wts[0][0][:], in_=wds[0][0][:])
            # row-piece boundaries matched to the slot-0 compute phases
            # (chunks 0-1 read rows 0-17, 2-3 rows 16-33, 4-6 rows 32-57)
            rb = [0, 18, 34] if m0 == NCHUNK else [0]
            pieces = [r * PW for r in rb] + [cols0]
            # interleaved layout: piece [c0, c1) in cols = contiguous bytes
            for c0, c1 in zip(pieces, pieces[1:]):
                nc.sync.dma_start(out=xts[0][0][:, 2 * c0:2 * c1],
                                  in_=xds[0][0][:, 2 * c0:2 * c1])
            for i in range(1, NW):
                nc.scalar.dma_start(out=wts[0][i][:], in_=wds[0][i][:])
            for i in range(1, NX):
                nc.scalar.dma_start(out=xts[0][i][:], in_=xds[0][i][:])
            # later slots' inputs are deferred until the first matmul has
            # run: at t=0 all 8 cores burst their full input set at HBM
            # and the critical slot-0 completion receipts (which gate the
            # first matmul) pay the high-load latency.  Emitted after the
            # slot-0 compute with a sync dep on the first matmul.
            deferred = []
            for s in range(1, len(template)):
                ring = nc.scalar if s % 2 == 1 else nc.sync
                for i in range(NW):
                    deferred.append((ring, wts[s][i], wds[s][i]))
                for i in range(NX):
                    deferred.append((ring, xts[s][i], xds[s][i]))

            prev_mm = [None]
            anchor = [None]

            def mm_chain(inst):
                if DEDUP_LDW:
                    if prev_mm[0] is not None:
                        add_dep_helper(inst.ins, prev_mm[0].ins, sync=False,
                                       reason="pe-order for ldweights dedup")
                    prev_mm[0] = inst

            # warm the PE (p-state ramp) while slot-0 data lands; memset-fed
            # so it starts as soon as the engines come alive, sized to cover
            # the first-piece DMA latency (~1.5us)
            wsrc = pw.tile([128, 512], FP8, tag="wsrc")
            nc.vector.memset(wsrc[:], 0.0)
            wsv = wsrc[:].rearrange("p (two j) -> p two j", two=2)
            wps = pp.tile([128, 256], F32, tag="warmps")
            for _ in range(14):
                w = nc.tensor.matmul(wps[:], wsv[:, :, 0:128], wsv[:],
                                     start=True, stop=True, perf_mode=DR,
                                     skip_group_check=True)
                mm_chain(w)

            npass = len(PASSES)
            for s, m in enumerate(template):
                banks = [pp.tile([128, CH_N], F32, tag=f"bank{k}",
                                 name=f"bank{s}_{k}")
                         for k in range(m)]
                # slot 0 runs in chunk-range phases matched to the xh piece
                # boundaries, so compute never outruns the wire
                if s == 0 and m == NCHUNK:
                    kranges = [(0, 2), (2, 4), (4, m)]
                else:
                    kranges = [(0, m)]
                for klo, khi in kranges:
                    for g in range(9 * npass):
                        p, tap = divmod(g, 9)
                        wsel, xsel = PASSES[p]
                        dy, dx = tap // 3, tap % 3
                        for k in range(klo, khi):
                            off = (CH_ROWS * k + dy) * PW + dx
                            w = nc.tensor.matmul(
                                banks[k][:, 0:CH_N - 2],
                                wvs[s][wsel][:, tap],
                                xvs[s][xsel][:, :, off:off + CH_N - 2],
                                start=(g == 0), stop=(g == 9 * npass - 1),
                                perf_mode=DR, skip_group_check=True)
                            mm_chain(w)
                            if anchor[0] is None:
                                anchor[0] = w
                if s == 0:
                    for ring, o, i_ in deferred:
                        dd = ring.dma_start(out=o[:], in_=i_[:])
                        add_dep_helper(dd.ins, anchor[0].ins, sync=True,
                                       reason="defer noncritical input DMA "
                                              "past the first matmul")

                stg = pw.tile([128, m * CH_ROWS * W], BF16, tag=f"stg{s}")
                sv = stg[:].rearrange("p (r c) -> p r c", c=W)
                for k in range(m):
                    bv = banks[k][:].rearrange("p (r c) -> p r c", c=PW)
                    if m == 1:
                        # program-tail chunk: split across both engines
                        half = CH_ROWS // 2
                        nc.scalar.activation(
                            sv[:, 0:half, :], bv[:, 0:half, 0:W],
                            AF.Copy, scale=1.0 / WSCALE)
                        nc.vector.tensor_scalar(
                            out=sv[:, half:CH_ROWS, :],
                            in0=bv[:, half:CH_ROWS, 0:W],
                            scalar1=1.0 / WSCALE, scalar2=None, op0=ALU.mult)
                    elif k % 2 == 0:
                        nc.scalar.activation(
                            sv[:, k * CH_ROWS:(k + 1) * CH_ROWS, :],
                            bv[:, :, 0:W], AF.Copy, scale=1.0 / WSCALE)
                    else:
                        nc.vector.tensor_scalar(
                            out=sv[:, k * CH_ROWS:(k + 1) * CH_ROWS, :],
                            in0=bv[:, :, 0:W],
                            scalar1=1.0 / WSCALE, scalar2=None, op0=ALU.mult)
                # gpsimd SWDGE ring (otherwise idle) for mid-program
                # outputs; two pieces so the first chunks' rows stream out
                # while the tail chunks are still extracting.  The last
                # slot's output takes the (by then idle) sync HWDGE ring:
                # lower first-byte latency on the program's critical tail.
                last = s == len(template) - 1
                ring = nc.sync if last else nc.gpsimd
                if m > 4:
                    cmid = 4 * CH_ROWS * W
                    ring.dma_start(out=outds[s][:, 0:cmid],
                                   in_=stg[:, 0:cmid])
                    ring.dma_start(out=outds[s][:, cmid:],
                                   in_=stg[:, cmid:])
                else:
                    ring.dma_start(out=outds[s][:], in_=stg[:])

    if DEDUP_LDW:
        _dedup_ldweights(nc)
    nc.compile()
    return nc


def _build(template):
    if USE_FP8:
        return _build_fp8(template)
    nc = bacc.Bacc(None, target_bir_lowering=False)

    xins, wsls, outds = [], [], []
    for s, m in enumerate(template):
        cols = (8 * m + 2) * PW + TAIL
        xins.append([nc.declare_dram_parameter(
            f"xin{s}_{kb}", [128, cols], BF16, isOutput=False)
            for kb in range(2)])
        wsls.append([nc.declare_dram_parameter(
            f"wsl{s}_{kb}", [128, 9 * 128], BF16, isOutput=False)
            for kb in range(2)])
        outds.append(nc.declare_dram_parameter(
            f"outd{s}", [128, m * CH_ROWS * W], BF16, isOutput=True))

    with TileContext(nc) as tc:
        with tc.tile_pool(name="work", bufs=1) as pw, \
             tc.tile_pool(name="psum", bufs=1, space="PSUM") as pp:

            wts, xbs = [], []
            for s, m in enumerate(template):
                cols = (8 * m + 2) * PW + TAIL
                wts.append([pw.tile([128, 9 * 128], BF16, tag=f"w{s}_{kb}",
                                    name=f"w{s}_{kb}")
                            for kb in range(2)])
                xbs.append([pw.tile([128, cols], BF16, tag=f"x{s}_{kb}",
                                    name=f"x{s}_{kb}")
                            for kb in range(2)])

            # DMA emission in criticality order: queues drain descriptors in
            # instruction order, so slot 0's kb=0 data (needed by the first
            # 9 matmul groups) comes first.  xin0_0 is split into row-pieces:
            # group 0's k-loop walks rows bottom-up, and range-level hazard
            # tracking lets chunk k's matmul start when its piece lands.
            m0 = template[0]
            nrow0 = 8 * m0 + 2
            # boundaries aligned to the two slot-0 compute phases
            # (chunks 0-3 read rows 0-33, chunks 4-6 read rows 32-57)
            if m0 == NCHUNK:
                rb = [0, 18, 34, nrow0]
            else:
                rb = [0, nrow0]
            pieces = [r * PW for r in rb[:-1]] + [nrow0 * PW + TAIL]
            # only wsl0_0's tap-0 slice (32 KB) gates the first matmul; the
            # other taps are consumed over the next 12 us, so they stream
            # behind the image whose completion bounds the kb0 phase
            nc.sync.dma_start(out=wts[0][0][:, 0:128], in_=wsls[0][0][:, 0:128])
            for c0, c1 in zip(pieces, pieces[1:]):
                nc.sync.dma_start(out=xbs[0][0][:, c0:c1],
                                  in_=xins[0][0][:, c0:c1])
            nc.sync.dma_start(out=wts[0][0][:, 128:], in_=wsls[0][0][:, 128:])
            nc.sync.dma_start(out=wts[0][1][:], in_=wsls[0][1][:])
            nc.sync.dma_start(out=xbs[0][1][:], in_=xins[0][1][:])
            for s in range(1, len(template)):
                for kb in range(2):
                    nc.sync.dma_start(out=wts[s][kb][:], in_=wsls[s][kb][:])
                    nc.sync.dma_start(out=xbs[s][kb][:], in_=xins[s][kb][:])

            # warm the PE (p-state ramp) while slot-0 data lands; memset-fed
            # so the warm-up starts as soon as the engines come alive, and
            # long enough (~6us) that the PE does not idle-reset its ramp
            # before the first conv matmul's data arrives
            wsrc = pw.tile([128, 256], BF16, tag="wsrc")
            nc.vector.memset(wsrc[:], 0.0)
            wps = pp.tile([128, 256], F32, tag="warmps")
            for _ in range(24):
                nc.tensor.matmul(wps[:, 0:128], wsrc[:, 0:128], wsrc[:, 0:128],
                                 start=True, stop=True, skip_group_check=True)
            for _ in range(6):
                nc.tensor.matmul(wps[:], wsrc[:, 0:128], wsrc[:],
                                 start=True, stop=True, skip_group_check=True)

            for s, m in enumerate(template):
                banks = [pp.tile([128, CH_N], F32, tag=f"bank{k}",
                                 name=f"bank{s}_{k}")
                         for k in range(m)]
                # slot 0 kb0 runs in two chunk-range phases matched to the
                # xin piece boundaries, so compute never outruns the wire
                if s == 0 and m == NCHUNK:
                    kranges = [(0, 4), (4, NCHUNK)]
                else:
                    kranges = [(0, m)]
                for klo, khi in kranges:
                    for g in range(9):
                        tap = g
                        dy, dx = tap // 3, tap % 3
                        for k in range(klo, khi):
                            off = (CH_ROWS * k + dy) * PW + dx
                            nc.tensor.matmul(
                                banks[k][:, 0:CH_N - 2],
                                wts[s][0][:, tap * 128:(tap + 1) * 128],
                                xbs[s][0][:, off:off + CH_N - 2],
                                start=(g == 0), stop=False,
                                skip_group_check=True)
                for g in range(9, 18):
                    tap = g - 9
                    dy, dx = tap // 3, tap % 3
                    for k in range(m):
                        off = (CH_ROWS * k + dy) * PW + dx
                        nc.tensor.matmul(
                            banks[k][:, 0:CH_N - 2],
                            wts[s][1][:, tap * 128:(tap + 1) * 128],
                            xbs[s][1][:, off:off + CH_N - 2],
                            start=False, stop=(g == 17),
                            skip_group_check=True)

                stg = pw.tile([128, m * CH_ROWS * W], BF16, tag=f"stg{s}")
                sv = stg[:].rearrange("p (r c) -> p r c", c=W)
                for k in range(m):
                    bv = banks[k][:].rearrange("p (r c) -> p r c", c=PW)
                    if k % 2 == 0:
                        nc.scalar.activation(
                            sv[:, k * CH_ROWS:(k + 1) * CH_ROWS, :],
                            bv[:, :, 0:W], AF.Copy)
                    else:
                        nc.vector.tensor_copy(
                            out=sv[:, k * CH_ROWS:(k + 1) * CH_ROWS, :],
                            in_=bv[:, :, 0:W])
                # two pieces so the first chunks' rows stream out while the
                # tail chunks are still being extracted; sync ring is idle
                # by output time (scalar stays copy-only)
                if m > 4:
                    cmid = 4 * CH_ROWS * W
                    nc.sync.dma_start(out=outds[s][:, 0:cmid],
                                      in_=stg[:, 0:cmid])
                    nc.sync.dma_start(out=outds[s][:, cmid:],
                                      in_=stg[:, cmid:])
                else:
                    nc.sync.dma_start(out=outds[s][:], in_=stg[:])

    nc.compile()
    return nc


# ---------------------------------------------------------------- packing

def _pack_inputs_fp8(inputs, template, assign):
    x = np.asarray(inputs["x"], np.float32)
    conv_w = np.asarray(inputs["conv_w"], np.float32)

    # per-sample padded fp8 hi/lo image pair, built lazily
    padded = {}

    def pimg(b):
        if b not in padded:
            p = np.zeros((C, PH, PW), np.float32)
            p[:, 1:57, 1:57] = x[b]
            hi = p.astype(E4_NP)
            lo = (p - hi.astype(np.float32)).astype(E4_NP) if NEED_XL else None
            padded[b] = (hi, lo)
        return padded[b]

    # per-unit hi/lo slabs: slab[p, tap*256 + kb*128 + j] =
    # WSCALE*conv_w[sel_j, kb*128 + p, dy, dx], fp8-split
    slabs = {}

    def slab(b, ids):
        key = (b, ids.tobytes())
        if key not in slabs:
            sl = np.zeros((128, 9 * 256), np.float32)
            wsel = conv_w[ids] * WSCALE              # [n, C, 3, 3]
            n = len(ids)
            for tap in range(9):
                dy, dx = tap // 3, tap % 3
                for kb in range(2):
                    col = tap * 256 + kb * 128
                    sl[:, col:col + n] = \
                        wsel[:, kb * 128:(kb + 1) * 128, dy, dx].T
            hi = sl.astype(E4_NP)
            lo = (sl - hi.astype(np.float32)).astype(E4_NP) if NEED_WL else None
            slabs[key] = (hi, lo)
        return slabs[key]

    in_maps = []
    for core in range(NCORES):
        m_map = {}
        for s, m in enumerate(template):
            cols = _ccols(m)
            xh = np.zeros((128, 2 * cols), E4_NP)
            xl = np.zeros((128, 2 * cols), E4_NP) if NEED_XL else None
            wh = np.zeros((128, 9 * 256), E4_NP)
            wl = np.zeros((128, 9 * 256), E4_NP) if NEED_WL else None
            a = assign[core][s]
            if a is not None:
                b, ids, a0, _, _ = a
                hi, lo = pimg(b)
                nrow = 8 * m + 2
                rh = hi[:, 8 * a0:8 * a0 + nrow, :].reshape(C, -1)
                nr = rh.shape[1]
                # interleave the cin pair (c, c+128) byte-adjacent:
                # xh[p, 2*i + kb] = x[kb*128 + p, i]
                xhv = xh.reshape(128, cols, 2)
                for kb in range(2):
                    xhv[:, :nr, kb] = rh[kb * 128:(kb + 1) * 128]
                if NEED_XL:
                    rl = lo[:, 8 * a0:8 * a0 + nrow, :].reshape(C, -1)
                    xlv = xl.reshape(128, cols, 2)
                    for kb in range(2):
                        xlv[:, :nr, kb] = rl[kb * 128:(kb + 1) * 128]
                whs, wls = slab(b, ids)
                wh[:] = whs
                if NEED_WL:
                    wl[:] = wls
            m_map[f"xh{s}"] = xh
            m_map[f"wh{s}"] = wh
            if NEED_XL:
                m_map[f"xl{s}"] = xl
            if NEED_WL:
                m_map[f"wl{s}"] = wl
        in_maps.append(m_map)
    return in_maps


def _pack_inputs(inputs, template, assign):
    if USE_FP8:
        return _pack_inputs_fp8(inputs, template, assign)
    x = np.asarray(inputs["x"], np.float32)
    conv_w = np.asarray(inputs["conv_w"], np.float32)

    # per-sample padded bf16 image, built lazily
    padded = {}

    def pimg(b):
        if b not in padded:
            p = np.zeros((C, PH, PW), np.float32)
            p[:, 1:57, 1:57] = x[b]
            padded[b] = p.astype(BF16_NP)
        return padded[b]

    # per-unit weight slabs, built lazily:  slab[kb][cin, tap*128 + i] =
    # conv_w[sel_i, kb*128 + cin, dy, dx]
    slabs = {}

    def slab(b, ids):
        key = (b, ids.tobytes())
        if key not in slabs:
            sl = np.zeros((2, 128, 9 * 128), np.float32)
            wsel = conv_w[ids]                       # [n, C, 3, 3]
            n = len(ids)
            for tap in range(9):
                dy, dx = tap // 3, tap % 3
                for kb in range(2):
                    sl[kb, :, tap * 128:tap * 128 + n] = \
                        wsel[:, kb * 128:(kb + 1) * 128, dy, dx].T
            slabs[key] = sl.astype(BF16_NP)
        return slabs[key]

    in_maps = []
    for core in range(NCORES):
        m_map = {}
        for s, m in enumerate(template):
            cols = (8 * m + 2) * PW + TAIL
            xin = np.zeros((2, 128, cols), BF16_NP)
            wsl = np.zeros((2, 128, 9 * 128), BF16_NP)
            a = assign[core][s]
            if a is not None:
                b, ids, a0, _, _ = a
                rows = pimg(b)[:, 8 * a0:8 * a0 + 8 * m + 2, :] \
                    .reshape(C, -1)                  # [C, (8m+2)*58]
                xin[0, :, :rows.shape[1]] = rows[:128]
                xin[1, :, :rows.shape[1]] = rows[128:]
                wsl[:] = slab(b, ids)
            for kb in range(2):
                m_map[f"xin{s}_{kb}"] = xin[kb]
                m_map[f"wsl{s}_{kb}"] = wsl[kb]
        in_maps.append(m_map)
    return in_maps


def _assemble(inputs, template, assign, results):
    x = np.asarray(inputs["x"], np.float32)
    out = x.copy()
    for core in range(NCORES):
        for s, m in enumerate(template):
            a = assign[core][s]
            if a is None:
                continue
            b, ids, a0, r0, r1 = a
            n = len(ids)
            data = np.asarray(results[core][f"outd{s}"]) \
                .reshape(128, m * CH_ROWS, W)[:n].astype(np.float32)
            lk0, lk1 = r0 - a0, r1 - a0
            out[b, ids, 8 * r0:8 * r1, :] = \
                data[:, lk0 * CH_ROWS:lk1 * CH_ROWS, :]
    return out


# ---------------------------------------------------------------- entry

def kernel(**inputs):
    sel = _host_gates(inputs)
    template, assign = _schedule(sel)

    tkey = tuple(template)
    if _CACHE.get("tkey") != tkey:
        _CACHE["nc"] = _build(template)
        _CACHE["tkey"] = tkey
    nc = _CACHE["nc"]

    in_maps = _pack_inputs(inputs, template, assign)

    trace = bool(int(os.environ.get("BASS_KERNEL_TRACE", "0")))
    kw = {}
    if trace:
        from trn_agent_boot.trn_boot import _ntff_profile_via_ctypes
        import antenv.axon_hooks as ah
        ah.set_axon_ntff_profile_hook(
            _ntff_profile_via_ctypes("/opt/axon/libaxon_pjrt.so"))
        import tempfile
        base = os.environ.get("BASS_KERNEL_TRACE_DIR", "/tmp/adaptconv_trace")
        os.makedirs(base, exist_ok=True)
        kw = dict(trace=True, tmpdir=tempfile.mkdtemp(dir=base))

    res = run_bass_kernel_spmd(nc, in_maps, core_ids=list(range(NCORES)), **kw)
    _CACHE["last_exec_time_ns"] = res.exec_time_ns

    return _assemble(inputs, template, assign, res.results)

